# revision 15
# baseline (speedup 1.0000x reference)
"""Trainium2 Bass kernel for nn_Attention_91061896610676.

Math (verified against reference to ~3e-6 rel err):
  z = stack([f1..f4], 1); q,k,v = z@Wq+bq, z@Wk+bk, z@Wv+bv
  score = softmax(q @ k^T);  x = sum_t (score @ v)_t;  out = softmax(x@Wfc+bfc)

Algebraic reduction used here:
  d[b,t,s] = q_t.k_s = f_t M f_s^T + f_t.(Wq bk) + f_s.(Wk bq) + bq.bk
  with M = Wq Wk^T.  Softmax over s is invariant to terms constant in s, so
  softmax_s(d) == softmax_s(ghat_t . f_s) with ghat_t = f_t M + (Wk bq).
  wgt[b,s] = sum_t softmax_s(...)[t,s];  sum_s wgt = 4
  x = (sum_s wgt_s f_s) @ Wv + 4 bv          (V projection collapses to 1 matmul)
  out = softmax(y @ (Wv Wfc) + (4 bv Wfc + bfc))

Per-core layout (pure data parallel over batch, 8 cores x 2048 rows):
  - ghat^T computed directly in [h, b] layout (stationary M chunks);
    scores via PE Gram matmuls + DVE diagonal extraction (tensor_tensor_reduce
    against identity); y accumulated transposed; x/logits from yT with
    stationary yT chunks.  float32r (TF32-like fast path) for all matmuls.
"""

import numpy as np

B, H, OUT = 16384, 1024, 128
NCORES = 8
BLOC = B // NCORES          # 2048 rows per core
SUPER = 256                 # supertile rows
NSUB = SUPER // 128         # 2
NCH = H // 128              # 8 feature chunks
WVE_W = H + 2 * OUT         # Wv | Wvfc | zero-pad  -> 1280 cols

_CACHE = {}
TRACE = False       # set True (e.g. from test.py) to capture a neuron profile
LAST = {}           # run metadata (exec_time_ns) when TRACE is on


def _build(nsup):
    """Build the per-core Bass program processing nsup supertiles."""
    import concourse.bass as bass
    import concourse.mybir as mybir
    from concourse import bacc
    from concourse.tile import TileContext

    f32 = mybir.dt.float32
    f32r = mybir.dt.float32r
    AF = mybir.ActivationFunctionType
    ALU = mybir.AluOpType
    AX = mybir.AxisListType

    rows = nsup * SUPER

    nc = bacc.Bacc(None, target_bir_lowering=False)
    f_ext = [
        nc.declare_dram_parameter(f"f{i + 1}", [rows, H], f32, isOutput=False)
        for i in range(4)
    ]
    M_ext = nc.declare_dram_parameter("Mw", [H, H], f32, isOutput=False)
    Wve_ext = nc.declare_dram_parameter("Wve", [H, WVE_W], f32, isOutput=False)
    w_ext = nc.declare_dram_parameter("wvec", [H, 1], f32, isOutput=False)
    bias_ext = nc.declare_dram_parameter("biasrow", [1, WVE_W], f32, isOutput=False)
    ones_ext = nc.declare_dram_parameter("onesrow", [1, 128], f32, isOutput=False)
    ident_ext = nc.declare_dram_parameter("ident", [128, 128], f32, isOutput=False)
    x_ext = nc.declare_dram_parameter("x", [rows, H], f32, isOutput=True)
    out_ext = nc.declare_dram_parameter("out", [rows, OUT], f32, isOutput=True)

    with TileContext(nc) as tc:
        with (
            tc.tile_pool(name="const", bufs=1) as cpool,
            tc.tile_pool(name="stage", bufs=2) as stpool,
            tc.tile_pool(name="ps", bufs=1, space="PSUM") as pspool,
        ):
            # ---------------- constants ----------------
            M_r = cpool.tile([128, NCH, H], f32r, name="M_r")
            Wve_r = cpool.tile([128, NCH, WVE_W], f32r, name="Wve_r")
            w_sb = cpool.tile([128, NCH], f32, name="w_sb")
            ident = cpool.tile([128, 128], f32, name="ident")
            ones_r = cpool.tile([1, 128], f32r, name="ones_r")
            bias_r = cpool.tile([1, WVE_W], f32r, name="bias_r")

            nc.sync.dma_start(ident[:], ident_ext[:])
            nc.sync.dma_start(w_sb[:], w_ext.rearrange("(c p) o -> p (c o)", p=128))
            with tc.tile_pool(name="setup", bufs=2) as supool:
                for c in range(NCH):
                    tmp = supool.tile([128, H], f32, name="tmp", tag="tmp")
                    nc.sync.dma_start(tmp[:], M_ext[c * 128:(c + 1) * 128, :])
                    nc.vector.tensor_copy(M_r[:, c, :], tmp[:])
                for c in range(NCH):
                    tmp = supool.tile([128, WVE_W], f32, name="tmpv", tag="tmp")
                    nc.sync.dma_start(tmp[:], Wve_ext[c * 128:(c + 1) * 128, :])
                    nc.vector.tensor_copy(Wve_r[:, c, :], tmp[:])
                tmpo = supool.tile([1, 128], f32, name="tmpo", tag="small")
                nc.sync.dma_start(tmpo[:], ones_ext[:])
                nc.vector.tensor_copy(ones_r[:], tmpo[:])
                tmpb = supool.tile([1, WVE_W], f32, name="tmpb", tag="small")
                nc.sync.dma_start(tmpb[:], bias_ext[:])
                nc.vector.tensor_copy(bias_r[:], tmpb[:])
            wstack = __import__("contextlib").ExitStack()
            wpool = wstack.enter_context(tc.tile_pool(name="work", bufs=1))

            # ---------------- main loop ----------------
            for st in range(nsup):
                r0 = st * SUPER

                # fT: transposed activations [h within chunk, chunk, token, b]
                fT = wpool.tile([128, NCH, 4, SUPER], f32r, name=f"fT{st}",
                                tag="fT", bufs=1)
                for t in range(4):
                    for sub in range(NSUB):
                        fn = wpool.tile([128, H], f32, name=f"fn{st}{t}{sub}",
                                        tag="fnat", bufs=3)
                        nc.sync.dma_start(
                            fn[:], f_ext[t][r0 + sub * 128: r0 + (sub + 1) * 128, :])
                        for half in range(2):
                            tps = pspool.tile([128, 512], f32, name=f"tp{st}{t}{sub}{half}",
                                              tag="psA", bufs=2)
                            for q in range(4):
                                ch = half * 4 + q
                                nc.tensor.transpose(
                                    tps[:, q * 128:(q + 1) * 128],
                                    fn[:, ch * 128:(ch + 1) * 128], ident[:])
                            nc.vector.tensor_copy(
                                fT[:, half * 4:(half + 1) * 4, t,
                                   sub * 128:(sub + 1) * 128],
                                tps.rearrange("p (c b) -> p c b", c=4))

                # ghat^T then Gram, interleaved per token pair
                dtile = wpool.tile([128, NSUB, 16], f32, name=f"d{st}",
                                   tag="dtile", bufs=2)
                for th in (0, 2):          # token halves: 2 concurrent PSUM groups
                    gT = {t: wpool.tile([128, NCH, SUPER], f32r, name=f"gT{st}{t}",
                                        tag=f"gT{t % 2}", bufs=1)
                          for t in (th, th + 1)}
                    gps = {}
                    for hout in range(NCH):
                        for kc in range(NCH):
                            for t in (th, th + 1):
                                if kc == 0:
                                    gps[t] = pspool.tile([128, SUPER], f32,
                                                         name=f"gp{st}{hout}{t}",
                                                         tag="psB", bufs=2)
                                nc.tensor.matmul(
                                    gps[t][:],
                                    M_r[:, kc, hout * 128:(hout + 1) * 128],
                                    fT[:, kc, t, :],
                                    start=(kc == 0), stop=(kc == NCH - 1))
                        for t in (th, th + 1):
                            # copy + bias(w) + round to f32r on ScalarE
                            nc.scalar.activation(gT[t][:, hout, :], gps[t][:],
                                                 AF.Identity,
                                                 bias=w_sb[:, hout:hout + 1],
                                                 scale=1.0)
                    for t in (th, th + 1):
                        for sub in range(NSUB):
                            sps = pspool.tile([128, 512], f32, name=f"sp{st}{t}{sub}",
                                              tag="psC", bufs=2)
                            for kc in range(NCH):
                                nc.tensor.matmul(
                                    sps[:],
                                    gT[t][:, kc, sub * 128:(sub + 1) * 128],
                                    fT[:, kc, :, sub * 128:(sub + 1) * 128],
                                    start=(kc == 0), stop=(kc == NCH - 1))
                            # diag extract: mask by identity then reduce per s-block
                            scr = wpool.tile([128, 512], f32,
                                             name=f"scr{st}{t}{sub}",
                                             tag="scr", bufs=2)
                            nc.vector.tensor_tensor(
                                scr.rearrange("p (s b) -> p s b", s=4),
                                sps.rearrange("p (s b) -> p s b", s=4),
                                ident[:].unsqueeze(1).broadcast_to([128, 4, 128]),
                                ALU.mult)
                            nc.vector.tensor_reduce(
                                dtile[:, sub, t * 4:(t + 1) * 4],
                                scr.rearrange("p (s b) -> p s b", s=4),
                                axis=AX.X, op=ALU.add)

                # softmax over s; wgt[b, s] = sum_t P[t, s]
                wgt = wpool.tile([128, NSUB, 4], f32, name=f"wgt{st}",
                                 tag="wgt", bufs=2)
                for sub in range(NSUB):
                    dv = dtile[:, sub, :].rearrange("p (t s) -> p t s", t=4)
                    mx = wpool.tile([128, 4], f32, name=f"mx{st}{sub}", tag="mx", bufs=2)
                    nc.vector.tensor_reduce(mx[:], dv, axis=AX.X, op=ALU.max)
                    es = wpool.tile([128, 16], f32, name=f"es{st}{sub}", tag="es", bufs=2)
                    nc.vector.tensor_tensor(
                        es.rearrange("p (t s) -> p t s", t=4), dv,
                        mx[:].unsqueeze(2).broadcast_to([128, 4, 4]),
                        mybir.AluOpType.subtract)
                    nc.scalar.activation(es[:], es[:], AF.Exp)
                    sm = wpool.tile([128, 4], f32, name=f"sm{st}{sub}", tag="sm", bufs=2)
                    nc.vector.tensor_reduce(
                        sm[:], es.rearrange("p (t s) -> p t s", t=4),
                        axis=AX.X, op=ALU.add)
                    rs = wpool.tile([128, 4], f32, name=f"rs{st}{sub}", tag="rs", bufs=2)
                    nc.vector.reciprocal(rs[:], sm[:])
                    nc.vector.tensor_tensor(
                        es.rearrange("p (t s) -> p t s", t=4),
                        es.rearrange("p (t s) -> p t s", t=4),
                        rs[:].unsqueeze(2).broadcast_to([128, 4, 4]),
                        mybir.AluOpType.mult)
                    # reduce over t: view as [p, s, t] (s stride 1, t stride 4)
                    nc.vector.tensor_reduce(
                        wgt[:, sub, :],
                        es.rearrange("p (t s) -> p s t", t=4),
                        axis=AX.X, op=ALU.add)

                # wgt^T rows on partition 0, then wrep via PE outer product
                wtps = [pspool.tile([128, 512], f32, name=f"wtp{st}{i}",
                                    tag="psA", bufs=2) for i in range(2)]
                for s in range(4):
                    for sub in range(NSUB):
                        nc.tensor.transpose(
                            wtps[s // 2][0:1, (s % 2) * 256 + sub * 128:
                                         (s % 2) * 256 + (sub + 1) * 128],
                            wgt[:, sub, s:s + 1], ident[:])
                wt1t = wpool.tile([1, 4 * SUPER], f32r, name=f"wt1{st}",
                                  tag="wt1", bufs=1)
                wt1 = [wt1t[0:1, s * SUPER:(s + 1) * SUPER] for s in range(4)]
                for s in range(4):
                    nc.vector.tensor_copy(
                        wt1[s], wtps[s // 2][0:1, (s % 2) * 256:(s % 2) * 256 + SUPER])
                wrp = pspool.tile([128, 4, SUPER], f32, name=f"wrp{st}", tag="psD", bufs=1)
                wrpf = wrp.rearrange("p a b -> p (a b)")
                for s in range(4):
                    nc.tensor.matmul(wrpf[:, s * SUPER:(s + 1) * SUPER],
                                     ones_r[:], wt1[s][:],
                                     start=True, stop=True)

                # yT accumulation: yT[h,b] = sum_s wrep_s * fT_s
                yT = wpool.tile([128, NCH, SUPER], f32r, name=f"yT{st}",
                                tag="yT", bufs=2)
                ytmp = wpool.tile([128, NCH, SUPER], f32, name=f"ytmp{st}",
                                  tag="ytmp", bufs=1)
                for s in range(4):
                    dst = yT[:] if s == 0 else ytmp[:]
                    nc.vector.tensor_tensor(
                        dst,
                        fT[:, :, s, :].bitcast(f32),
                        wrpf[:, s * SUPER:(s + 1) * SUPER].unsqueeze(1)
                        .broadcast_to([128, NCH, SUPER]),
                        mybir.AluOpType.mult)
                    if s > 0:
                        nc.vector.tensor_tensor(yT[:], yT[:].bitcast(f32),
                                                ytmp[:], mybir.AluOpType.add)

                # x | logits = yT^T @ (Wv|Wvfc|pad) + bias  per subtile
                for sub in range(NSUB):
                    xps = [pspool.tile([128, 512], f32, name=f"xp{st}{sub}{h}",
                                       tag="psB", bufs=2) for h in range(2)]
                    lps = pspool.tile([128, 256], f32, name=f"lp{st}{sub}",
                                      tag="psC", bufs=2)
                    for h in range(2):
                        for kc in range(NCH):
                            nc.tensor.matmul(
                                xps[h][:], yT[:, kc, sub * 128:(sub + 1) * 128],
                                Wve_r[:, kc, h * 512:(h + 1) * 512],
                                start=(kc == 0), stop=False)
                        nc.tensor.matmul(xps[h][:], ones_r[:],
                                         bias_r[:, h * 512:(h + 1) * 512],
                                         start=False, stop=True)
                    for kc in range(NCH):
                        nc.tensor.matmul(lps[:], yT[:, kc, sub * 128:(sub + 1) * 128],
                                         Wve_r[:, kc, 1024:1280],
                                         start=(kc == 0), stop=False)
                    nc.tensor.matmul(lps[:], ones_r[:], bias_r[:, 1024:1280],
                                     start=False, stop=True)

                    xst = stpool.tile([128, H], f32, name=f"xs{st}{sub}",
                                      tag="xst", bufs=2)
                    nc.vector.tensor_copy(xst[:, 0:512], xps[0][:])
                    nc.vector.tensor_copy(xst[:, 512:1024], xps[1][:])
                    nc.sync.dma_start(
                        x_ext[r0 + sub * 128: r0 + (sub + 1) * 128, :], xst[:])

                    # softmax over OUT=128 (first 128 of lps)
                    nmx = wpool.tile([128, 1], f32, name=f"nmx{st}{sub}",
                                     tag="nmx", bufs=2)
                    nc.vector.tensor_reduce(nmx[:], lps[:, 0:OUT], axis=AX.X,
                                            op=ALU.max, negate=True)
                    eo = wpool.tile([128, OUT], f32, name=f"eo{st}{sub}",
                                    tag="eo", bufs=2)
                    so = wpool.tile([128, 1], f32, name=f"so{st}{sub}",
                                    tag="so", bufs=2)
                    nc.scalar.activation(eo[:], lps[:, 0:OUT], AF.Exp,
                                         bias=nmx[:], scale=1.0, accum_out=so[:])
                    ro = wpool.tile([128, 1], f32, name=f"ro{st}{sub}",
                                    tag="ro", bufs=2)
                    nc.vector.reciprocal(ro[:], so[:])
                    ost = stpool.tile([128, OUT], f32, name=f"os{st}{sub}",
                                      tag="ost", bufs=2)
                    nc.vector.tensor_scalar_mul(ost[:], eo[:], ro[:])
                    nc.sync.dma_start(
                        out_ext[r0 + sub * 128: r0 + (sub + 1) * 128, :], ost[:])
            wstack.close()

    nc.compile()
    return nc


def _prep_weights(Wq, bq, Wk, bk, Wv, bv, Wfc, bfc):
    M = (Wq.astype(np.float64) @ Wk.astype(np.float64).T).astype(np.float32)
    wvec = (Wk.astype(np.float64) @ bq.astype(np.float64)).astype(np.float32)
    Wvfc = (Wv.astype(np.float64) @ Wfc.astype(np.float64)).astype(np.float32)
    bv4 = (4.0 * bv).astype(np.float32)
    bfch = (4.0 * (bv.astype(np.float64) @ Wfc.astype(np.float64))
            + bfc.astype(np.float64)).astype(np.float32)
    Wve = np.concatenate(
        [Wv.astype(np.float32), Wvfc,
         np.zeros((H, OUT), np.float32)], axis=1)          # [H, 1280]
    biasrow = np.concatenate(
        [bv4, bfch, np.zeros(OUT, np.float32)])[None, :]   # [1, 1280]
    return M, wvec[:, None], Wve, biasrow


def kernel(f1, f2, f3, f4, Wq, bq, Wk, bk, Wv, bv, Wfc, bfc):
    from concourse.bass_utils import run_bass_kernel_spmd

    f1, f2, f3, f4 = (np.ascontiguousarray(a, np.float32) for a in (f1, f2, f3, f4))
    M, wvec, Wve, biasrow = _prep_weights(
        *(np.asarray(a, np.float64) for a in (Wq, bq, Wk, bk, Wv, bv, Wfc, bfc)))
    onesrow = np.ones((1, 128), np.float32)
    ident = np.eye(128, dtype=np.float32)

    if "nc" not in _CACHE:
        _CACHE["nc"] = _build(BLOC // SUPER)
    nc = _CACHE["nc"]

    in_maps = []
    for i in range(NCORES):
        sl = slice(i * BLOC, (i + 1) * BLOC)
        in_maps.append({
            "f1": f1[sl], "f2": f2[sl], "f3": f3[sl], "f4": f4[sl],
            "Mw": M, "Wve": Wve, "wvec": wvec, "biasrow": biasrow,
            "onesrow": onesrow, "ident": ident,
        })
    res = run_bass_kernel_spmd(nc, in_maps, list(range(NCORES)), trace=TRACE)
    if TRACE:
        LAST["exec_time_ns"] = res.exec_time_ns
        LAST["mean_exec_time_ns"] = res.mean_exec_time_ns
        LAST["profile_json"] = res.profile_json
    x = np.concatenate([res.results[i]["x"] for i in range(NCORES)], axis=0)
    out = np.concatenate([res.results[i]["out"] for i in range(NCORES)], axis=0)
    return (x, out)
